# revision 13
# baseline (speedup 1.0000x reference)
"""GAT layer (nn_GATLayer) on 8 Trainium2 NeuronCores.

Math (per batch b):
    h   = x @ W                      [N, D]
    s1  = h @ a1   (free-dim i)      [N]
    s2  = h @ a2   (partition j)     [N]
    e   = lrelu(s1_i + s2_j)  masked by adj[i, j], softmax over j
    out = attn @ h

Device formulation (per core = one batch element), in [p=j, f=i] layout:
    t[j, i]  = select(A[j, i] > 0, max(y, 5y), -1e9),  y = s1[i] + s2[j]
               (custom DVE op; A is the uint8 adjacency -> 4MB/core DMA)
    p[j, i]  = exp(0.2 * t)                      (ACT, bf16 out)
    numT[d, i] = sum_j h_cat[j, d] * p[j, i],    h_cat = [h | ones]  (bf16)
    out[i, d]  = numT[d, i] / numT[64, i]

Sharding: data-parallel over batch B=8 across the 8 cores.

Host prep: x[b].T is shipped pre-cast to bf16 in a stacked [128, 1024]
layout (both column halves vertically, so DMAs cover 128 partitions);
the adjacency as uint8 adj.T; W duplicated as [W; W] (bf16 [128, 64]) and
[W.T | W.T] (f32 [64, 128]) so base-64 partition slices of xtb have
matching-base weights; a1/a2 packed as one [64, 2] tensor.

The steady loop is DVE-bound (custom score op, 2048 cols x ~1.15ns/col).
The prologue computes wa1/wa2 with ONE matmul, s1b via a broadcast-wa1
matmul straight from xtb, and each j-tile's h/s2 with one matmul against
[W | wa2], drained per group-of-4 into write-once hcat regions. The last
tile is split into halves so the exp/matmul tail pipelines.
"""

import os
import sys

sys.path.insert(0, "/opt/trn_rl_repo")

import numpy as np
import ml_dtypes

B, N, DIN, DOUT = 8, 2048, 64, 64
NCORES = 8
PJ = 128              # j-tile partition size
NJT = N // PJ         # 16 j-tiles
FCH = 512             # psum bank chunk (fp32)
NCH = N // FCH        # 4 chunks of the free dim
XQ = N // 2           # stacked xtb layout: [128, 1024]
NEG_BIG = -1.0e9
HCG = 4               # h/s2 tiles per psum group / hcat region
HCS = 66              # hcat stride: 64 h cols + 1 ones col + 1 pad
EPI_GRP = 4           # epilogue transposes packed per psum bank tile

_GAT_OP = None
_COMPILED = None
LAST_RESULT = None    # BassKernelResults from the last run (for test.py)


def _register_gat_op():
    """Fused score op: out = select(Src1 > 0, max(y, y*imm2), C1), y = Src0+C0.

    in0 = s1 broadcast [128, N] (f32), s0 = s2 per-partition [128, 1] (f32),
    in1 = adjacency tile [128, N] (uint8 0/1), s1 = -1e9, imm2 = 5.0.
    lrelu(x) = 0.2*max(5x, x); exp(0.2 * -1e9) -> 0 for masked entries.
    """
    global _GAT_OP
    if _GAT_OP is not None:
        return _GAT_OP
    from concourse.dve_ops import (
        OPS,
        CUSTOM_DVE_SPECS,
        DveOp,
        _SUB_OPCODE_FOR_NAME,
    )
    from concourse.dve_spec import (
        Spec, Src0, Src1, C0, C1, C2, Zero, maxx, select, lower, _has_src1,
    )
    from concourse.dve_uop import DveOpSpec

    name = "GAT_SCORE_U8_ANT"
    if name in _SUB_OPCODE_FOR_NAME:
        _GAT_OP = next(op for op in OPS if op.name == name)
        return _GAT_OP

    _y = Src0 + C0
    body = select(Src1 > Zero, maxx(_y, _y * C2), C1)

    def _ref(in0, in1, s0, s1, imm2):
        y = in0.astype(np.float32) + s0
        t = np.maximum(y, y * imm2)
        return np.where(in1.astype(np.float32) > 0.0, t, s1).astype(np.float32)

    spec = Spec(body=body, reference=_ref)
    row = max(_SUB_OPCODE_FOR_NAME.values()) + 1
    assert row < 0x20
    _SUB_OPCODE_FOR_NAME[name] = row
    shas = {}
    for ver in ("v3", "v4"):
        tmp = DveOpSpec(
            name=name, opcode=row, uops=lower(spec, ver=ver), rd1_en=_has_src1(spec)
        )
        shas[ver] = tmp.sha(ver)
    op = DveOp(name, spec, subdim=False, uops_sha=shas)
    OPS.append(op)
    CUSTOM_DVE_SPECS[name] = spec
    _GAT_OP = op
    return op


def _build_nc():
    """Build the Bass module (shared SPMD program for all 8 cores)."""
    from contextlib import ExitStack

    import concourse.bass as bass
    import concourse.tile as tile
    from concourse import bacc, masks, mybir

    gat_op = _register_gat_op()

    f32 = mybir.dt.float32
    bf16 = mybir.dt.bfloat16
    u8 = mybir.dt.uint8
    AF = mybir.ActivationFunctionType

    nc = bacc.Bacc("TRN2", target_bir_lowering=False, debug=False, num_devices=NCORES)

    # stacked bf16 x: rows 0-63 = x.T cols 0:1024, rows 64-127 = cols 1024:2048
    xtbd = nc.dram_tensor("xtb", [PJ, XQ], bf16, kind="ExternalInput").ap()
    mk = nc.dram_tensor("mask", [N, N], u8, kind="ExternalInput").ap()
    w2 = nc.dram_tensor("w2", [PJ, DOUT], bf16, kind="ExternalInput").ap()   # [W; W]
    wt2 = nc.dram_tensor("wt2", [DOUT, PJ], f32, kind="ExternalInput").ap()  # [W.T | W.T]
    a12 = nc.dram_tensor("a12", [DOUT, 2], f32, kind="ExternalInput").ap()   # [a1 | a2]
    out = nc.dram_tensor("out", [N, DOUT], f32, kind="ExternalOutput").ap()

    with ExitStack() as ctx:
        tc = ctx.enter_context(tile.TileContext(nc))

        const = ctx.enter_context(tc.tile_pool(name="const", bufs=1))
        big = ctx.enter_context(tc.tile_pool(name="big", bufs=1))

        # ---- input DMAs: one xtb chunk per dispatcher queue (each queue
        # serves its own list in order, striped over the 16 DMA engines).
        # The mask stream follows on the sync queue behind its xtb chunk.
        xtb = const.tile([PJ, XQ], bf16, tag="xtb")
        wrhs = const.tile([PJ, DOUT + 1], bf16, tag="wrhs")
        wt_dma = const.tile([DOUT, PJ], f32, tag="wt0")
        a12_dma = const.tile([DOUT, 2], f32, tag="a12d")
        TQ = XQ // 4  # 256-col quarters
        nc.sync.dma_start(wt_dma[:], wt2)
        nc.sync.dma_start(a12_dma[:], a12)
        nc.scalar.dma_start(wrhs[:, :DOUT], w2)
        nc.sync.dma_start(xtb[:, 0:TQ], xtbd[:, 0:TQ])
        nc.scalar.dma_start(xtb[:, TQ : 2 * TQ], xtbd[:, TQ : 2 * TQ])
        nc.gpsimd.dma_start(xtb[:, 2 * TQ : 3 * TQ], xtbd[:, 2 * TQ : 3 * TQ])
        nc.scalar.dma_start(xtb[:, 3 * TQ :], xtbd[:, 3 * TQ :])

        ones_sb = const.tile([PJ, 1], bf16, tag="ones")
        nc.vector.memset(ones_sb[:], 1.0)

        # psum pools: hpool and num_pool own their banks from the start so
        # the h-group matmuls never wait on recycled s1b banks.
        hpool = ctx.enter_context(tc.tile_pool(name="h_psum", bufs=1, space="PSUM"))
        num_pool = ctx.enter_context(
            tc.tile_pool(name="num_psum", bufs=1, space="PSUM")
        )

        with tc.tile_pool(name="wt_psum", bufs=1, space="PSUM") as wtpool:
            wa12_ps = wtpool.tile([PJ, 2], f32, tag="wa12_ps")
            nc.tensor.matmul(wa12_ps[:], wt_dma[:], a12_dma[:], start=True, stop=True)
            wa1rep = const.tile([PJ, PJ], bf16, tag="wa1rep")
            nc.scalar.copy(wa1rep[:], wa12_ps[:, 0:1].broadcast_to([PJ, PJ]))
            nc.scalar.copy(wrhs[:, DOUT : DOUT + 1], wa12_ps[:, 1:2])

        # ---- hcat regions (write-once, 4 tiles each) + per-group s2 cols
        hcat = [
            big.tile([PJ, HCG * HCS], bf16, name=f"hcat{g}", tag=f"hcat{g}")
            for g in range(NJT // HCG)
        ]
        s2g = [
            big.tile([PJ, HCG], f32, name=f"s2g{g}", tag=f"s2g{g}")
            for g in range(NJT // HCG)
        ]
        for g in range(NJT // HCG):
            h3 = hcat[g][:].rearrange("p (t s) -> p t s", s=HCS)
            nc.vector.tensor_copy(
                h3[:, :, DOUT : DOUT + 1],
                ones_sb[:].broadcast_to([PJ, HCG])[:, :, None],
            )

        def emit_hgroup(g):
            h_ps = hpool.tile([PJ, HCG * (DOUT + 1)], f32, tag="hps")
            for k in range(HCG):
                jt = g * HCG + k
                base = 0 if jt < 8 else DIN
                qoff = (jt % 8) * PJ
                nc.tensor.matmul(
                    h_ps[:, k * (DOUT + 1) : (k + 1) * (DOUT + 1)],
                    xtb[base : base + DIN, qoff : qoff + PJ],
                    wrhs[base : base + DIN, :],
                    start=True, stop=True,
                )
            h4 = h_ps[:].rearrange("p (t s) -> p t s", s=DOUT + 1)
            hc3 = hcat[g][:].rearrange("p (t s) -> p t s", s=HCS)
            nc.scalar.copy(hc3[:, :, :DOUT], h4[:, :, :DOUT])
            nc.scalar.copy(s2g[g][:], h4[:, :, DOUT])

        # group 0 first on PE: it unblocks the first score op's s2 column
        emit_hgroup(0)

        # ---- s1b = wa1rep.T @ xtb : [128, N] f32
        s1b_sb = big.tile([PJ, N], f32, tag="s1b")
        with tc.tile_pool(name="s1b_psum", bufs=3, space="PSUM") as spool:
            for c in range(NCH):
                base = 0 if c < 2 else DIN
                qsl = slice((c % 2) * FCH, (c % 2) * FCH + FCH)
                s1b_ps = spool.tile([PJ, FCH], f32, name=f"s1bps{c}", tag="s1b_ps")
                nc.tensor.matmul(
                    s1b_ps[:],
                    wa1rep[base : base + DIN, :],
                    xtb[base : base + DIN, qsl],
                    start=True, stop=True,
                )
                nc.vector.tensor_copy(s1b_sb[:, c * FCH : (c + 1) * FCH], s1b_ps[:])

        # ---- mask stream: pre-emit ALL tile DMAs; the sync queue serves
        # them in order behind its xtb chunk, so no explicit gating needed.
        # Tiles 8-15 wait on buffer-rotation WAR deps automatically.
        mpool = ctx.enter_context(tc.tile_pool(name="mask", bufs=8))
        mbs = []
        for jt in range(NJT):
            mb = mpool.tile([PJ, N], u8, name=f"mb{jt}", tag="mb")
            with tc.tile_wait_until(0.0010 + 0.00005 * jt):
                nc.sync.dma_start(mb[:], mk[jt * PJ : (jt + 1) * PJ, :])
            mbs.append(mb)

        emit_hgroup(1)

        tpool = ctx.enter_context(tc.tile_pool(name="scores", bufs=3))
        ppool_e = ctx.enter_context(tc.tile_pool(name="probs", bufs=3))
        numT = [
            num_pool.tile([DOUT + 1, FCH], f32, name=f"numt{c}", tag=f"numt{c}")
            for c in range(NCH)
        ]

        # ---- main loop over j-tiles (last tile split in half-columns so
        # the score/exp/matmul tail pipelines instead of serializing)
        for jt in range(NJT):
            g, k = jt // HCG, jt % HCG
            if jt in (HCG, 2 * HCG):
                emit_hgroup(g + 1)

            mb = mbs[jt]
            lhsT = hcat[g][:, k * HCS : k * HCS + DOUT + 1]
            t_sb = tpool.tile([PJ, N], f32, tag="t")
            p_sb = ppool_e.tile([PJ, N], bf16, tag="p")

            csls = [slice(0, N)] if jt < NJT - 1 else [
                slice(0, N // 2), slice(N // 2, N)
            ]
            for csl in csls:
                nc.vector._custom_dve(
                    gat_op,
                    out=t_sb[:, csl],
                    in0=s1b_sb[:, csl],
                    in1=mb[:, csl],
                    s0=s2g[g][:, k : k + 1],
                    s1=NEG_BIG,
                    imm2=5.0,
                )
                nc.scalar.activation(p_sb[:, csl], t_sb[:, csl], AF.Exp, scale=0.2)

            for c in range(NCH):
                sl = slice(c * FCH, (c + 1) * FCH)
                nc.tensor.matmul(
                    numT[c][:], lhsT, p_sb[:, sl],
                    start=(jt == 0), stop=(jt == NJT - 1),
                )

        # ---- epilogue: per 512-chunk: drain, transpose, divide, store ----
        ident0 = const.tile([PJ, PJ], f32, tag="ident0")
        masks.make_identity(nc, ident0[:])
        ident = const.tile([PJ, PJ], f32, tag="ident")
        nc.scalar.copy(ident[:], ident0[:])

        epool = ctx.enter_context(tc.tile_pool(name="epi", bufs=2))
        etr_pool = ctx.enter_context(
            tc.tile_pool(name="epi_psum", bufs=2, space="PSUM")
        )
        out_pool = ctx.enter_context(tc.tile_pool(name="out", bufs=1))

        out_sb = out_pool.tile([PJ, NJT * DOUT], f32, tag="out")
        out_3d = out.rearrange("(t p) d -> p t d", p=PJ)
        out_engines = [nc.sync, nc.scalar, nc.gpsimd, nc.sync]
        GW = EPI_GRP * (DOUT + 1)
        for g in range(NJT // EPI_GRP):
            numc = epool.tile([DOUT + 1, FCH], f32, tag="numc")
            if g % 2 == 0:
                nc.scalar.copy(numc[:], numT[g][:])
            else:
                nc.vector.tensor_copy(numc[:], numT[g][:])

            tr_ps = etr_pool.tile([PJ, GW], f32, tag="tr")
            for k in range(EPI_GRP):
                isl = slice(k * PJ, (k + 1) * PJ)
                nc.tensor.transpose(
                    tr_ps[:, k * (DOUT + 1) : (k + 1) * (DOUT + 1)],
                    numc[:, isl],
                    ident[: DOUT + 1, : DOUT + 1],
                )
            tr_sb = epool.tile([PJ, GW], f32, tag="tr_sb")
            nc.scalar.copy(tr_sb[:], tr_ps[:])

            tr3 = tr_sb[:].rearrange("p (k s) -> p k s", s=DOUT + 1)
            recip = epool.tile([PJ, EPI_GRP], f32, tag="recip")
            nc.vector.reciprocal(recip[:], tr3[:, :, DOUT])
            for k in range(EPI_GRP):
                it = g * EPI_GRP + k
                eng = (
                    nc.vector if g in (0, 3) else
                    (nc.gpsimd if g == 1 else nc.scalar)
                )
                if eng is nc.scalar:
                    nc.scalar.mul(
                        out_sb[:, it * DOUT : (it + 1) * DOUT],
                        tr3[:, k, :DOUT],
                        recip[:, k : k + 1],
                    )
                else:
                    eng.tensor_scalar_mul(
                        out_sb[:, it * DOUT : (it + 1) * DOUT],
                        tr3[:, k, :DOUT],
                        recip[:, k : k + 1],
                    )
            out_engines[g].dma_start(
                out_3d[:, g * EPI_GRP : (g + 1) * EPI_GRP, :],
                out_sb[:, g * EPI_GRP * DOUT : (g + 1) * EPI_GRP * DOUT].rearrange(
                    "p (t d) -> p t d", d=DOUT
                ),
            )

    nc.compile()
    return nc


def _prep_inputs(x, adj, W, a):
    xT = np.transpose(np.asarray(x, dtype=np.float32), (0, 2, 1))  # [B, 64, N]
    xtb = np.concatenate([xT[:, :, : N // 2], xT[:, :, N // 2 :]], axis=1)
    xtb = np.ascontiguousarray(xtb).astype(ml_dtypes.bfloat16)     # [B, 128, 1024]
    mask = np.ascontiguousarray(adj.T.astype(np.uint8))
    a = np.asarray(a, dtype=np.float32)
    a12 = np.ascontiguousarray(np.stack([a[:DOUT], a[DOUT:]], axis=1))  # [64, 2]
    W = np.asarray(W, dtype=np.float32)
    w2 = np.ascontiguousarray(np.concatenate([W, W], axis=0)).astype(
        ml_dtypes.bfloat16
    )                                                               # [128, 64]
    wt2 = np.ascontiguousarray(np.concatenate([W.T, W.T], axis=1))  # [64, 128]
    in_maps = []
    for b in range(NCORES):
        in_maps.append(
            {
                "xtb": xtb[b],
                "mask": mask,
                "w2": w2,
                "wt2": wt2,
                "a12": a12,
            }
        )
    return in_maps


def kernel(x, adj, W, a):
    global _COMPILED, LAST_RESULT
    from concourse import bass_utils

    x = np.asarray(x)
    adj = np.asarray(adj)
    assert x.shape == (B, N, DIN) and adj.shape == (N, N)

    if _COMPILED is None:
        _COMPILED = _build_nc()
    nc = _COMPILED

    in_maps = _prep_inputs(x, adj, W, a)
    res = bass_utils.run_bass_kernel_spmd(
        nc,
        in_maps,
        core_ids=list(range(NCORES)),
        trace=bool(int(os.environ.get("GAT_TRACE", "0"))),
    )
    LAST_RESULT = res
    out = np.stack([res.results[c]["out"] for c in range(NCORES)], axis=0)
    return out.astype(np.float32)


# revision 17
# speedup vs baseline: 1.0480x; 1.0480x over previous
"""GAT layer (nn_GATLayer) on 8 Trainium2 NeuronCores.

Math (per batch b):
    h   = x @ W                      [N, D]
    s1  = h @ a1   (free-dim i)      [N]
    s2  = h @ a2   (partition j)     [N]
    e   = lrelu(s1_i + s2_j)  masked by adj[i, j], softmax over j
    out = attn @ h

Device formulation (per core = one batch element), in [p=j, f=i] layout:
    t[j, i]  = select(A[j, i] > 0, max(y, 5y), -1e9),  y = s1[i] + s2[j]
               (custom DVE op; A is the uint8 adjacency -> 4MB/core DMA)
    p[j, i]  = exp(0.2 * t)                      (ACT, bf16 out)
    numT[d, i] = sum_j h_cat[j, d] * p[j, i],    h_cat = [h | ones]  (bf16)
    out[i, d]  = numT[d, i] / numT[64, i]

Sharding: data-parallel over batch B=8 across the 8 cores.

Host prep: x[b].T is shipped pre-cast to bf16 in a stacked [128, 1024]
layout (both column halves vertically, so DMAs cover 128 partitions);
the adjacency as uint8 adj.T; W duplicated as [W; W] (bf16 [128, 64]) and
[W.T | W.T] (f32 [64, 128]) so base-64 partition slices of xtb have
matching-base weights; a1/a2 packed as one [64, 2] tensor.

The steady loop is DVE-bound (custom score op, 2048 cols x ~1.15ns/col).
The prologue computes wa1/wa2 with ONE matmul, s1b via a broadcast-wa1
matmul straight from xtb, and each j-tile's h/s2 with one matmul against
[W | wa2], drained per group-of-4 into write-once hcat regions. The last
tile is split into halves so the exp/matmul tail pipelines.
"""

import os
import sys

sys.path.insert(0, "/opt/trn_rl_repo")

import numpy as np
import ml_dtypes

B, N, DIN, DOUT = 8, 2048, 64, 64
NCORES = 8
PJ = 128              # j-tile partition size
NJT = N // PJ         # 16 j-tiles
FCH = 512             # psum bank chunk (fp32)
NCH = N // FCH        # 4 chunks of the free dim
XQ = N // 2           # stacked xtb layout: [128, 1024]
NEG_BIG = -1.0e9
HCG = 4               # h/s2 tiles per psum group / hcat region
HCS = 66              # hcat stride: 64 h cols + 1 ones col + 1 pad
EPI_GRP = 4           # epilogue transposes packed per psum bank tile

_GAT_OP = None
_COMPILED = None
LAST_RESULT = None    # BassKernelResults from the last run (for test.py)


def _register_gat_op():
    """Fused score op: out = select(Src1 > 0, max(y, y*imm2), C1), y = Src0+C0.

    in0 = s1 broadcast [128, N] (f32), s0 = s2 per-partition [128, 1] (f32),
    in1 = adjacency tile [128, N] (uint8 0/1), s1 = -1e9, imm2 = 5.0.
    lrelu(x) = 0.2*max(5x, x); exp(0.2 * -1e9) -> 0 for masked entries.
    """
    global _GAT_OP
    if _GAT_OP is not None:
        return _GAT_OP
    from concourse.dve_ops import (
        OPS,
        CUSTOM_DVE_SPECS,
        DveOp,
        _SUB_OPCODE_FOR_NAME,
    )
    from concourse.dve_spec import (
        Spec, Src0, Src1, C0, C1, C2, Zero, maxx, select, lower, _has_src1,
    )
    from concourse.dve_uop import DveOpSpec

    name = "GAT_SCORE_U8_ANT"
    if name in _SUB_OPCODE_FOR_NAME:
        _GAT_OP = next(op for op in OPS if op.name == name)
        return _GAT_OP

    _y = Src0 + C0
    body = select(Src1 > Zero, maxx(_y, _y * C2), C1)

    def _ref(in0, in1, s0, s1, imm2):
        y = in0.astype(np.float32) + s0
        t = np.maximum(y, y * imm2)
        return np.where(in1.astype(np.float32) > 0.0, t, s1).astype(np.float32)

    spec = Spec(body=body, reference=_ref)
    row = max(_SUB_OPCODE_FOR_NAME.values()) + 1
    assert row < 0x20
    _SUB_OPCODE_FOR_NAME[name] = row
    shas = {}
    for ver in ("v3", "v4"):
        tmp = DveOpSpec(
            name=name, opcode=row, uops=lower(spec, ver=ver), rd1_en=_has_src1(spec)
        )
        shas[ver] = tmp.sha(ver)
    op = DveOp(name, spec, subdim=False, uops_sha=shas)
    OPS.append(op)
    CUSTOM_DVE_SPECS[name] = spec
    _GAT_OP = op
    return op


def _build_nc():
    """Build the Bass module (shared SPMD program for all 8 cores)."""
    from contextlib import ExitStack

    import concourse.bass as bass
    import concourse.tile as tile
    from concourse import bacc, masks, mybir

    gat_op = _register_gat_op()

    f32 = mybir.dt.float32
    bf16 = mybir.dt.bfloat16
    u8 = mybir.dt.uint8
    AF = mybir.ActivationFunctionType

    nc = bacc.Bacc("TRN2", target_bir_lowering=False, debug=False, num_devices=NCORES)

    # stacked bf16 x: rows 0-63 = x.T cols 0:1024, rows 64-127 = cols 1024:2048
    xtbd = nc.dram_tensor("xtb", [PJ, XQ], bf16, kind="ExternalInput").ap()
    mk = nc.dram_tensor("mask", [N, N], u8, kind="ExternalInput").ap()
    w2 = nc.dram_tensor("w2", [PJ, DOUT], bf16, kind="ExternalInput").ap()   # [W; W]
    wt2 = nc.dram_tensor("wt2", [DOUT, PJ], f32, kind="ExternalInput").ap()  # [W.T | W.T]
    a12 = nc.dram_tensor("a12", [DOUT, 2], f32, kind="ExternalInput").ap()   # [a1 | a2]
    out = nc.dram_tensor("out", [N, DOUT], bf16, kind="ExternalOutput").ap()

    with ExitStack() as ctx:
        tc = ctx.enter_context(tile.TileContext(nc))

        const = ctx.enter_context(tc.tile_pool(name="const", bufs=1))
        big = ctx.enter_context(tc.tile_pool(name="big", bufs=1))

        # ---- input DMAs: one xtb chunk per dispatcher queue (each queue
        # serves its own list in order, striped over the 16 DMA engines).
        # The mask stream follows on the sync queue behind its xtb chunk.
        xtb = const.tile([PJ, XQ], bf16, tag="xtb")
        wrhs = const.tile([PJ, DOUT + 1], bf16, tag="wrhs")
        wt_dma = const.tile([DOUT, PJ], f32, tag="wt0")
        a12_dma = const.tile([DOUT, 2], f32, tag="a12d")
        TQ = XQ // 4  # 256-col quarters
        nc.sync.dma_start(wt_dma[:], wt2)
        nc.sync.dma_start(a12_dma[:], a12)
        nc.scalar.dma_start(xtb[:, 0:TQ], xtbd[:, 0:TQ])
        nc.gpsimd.dma_start(xtb[:, TQ : 2 * TQ], xtbd[:, TQ : 2 * TQ])
        nc.scalar.dma_start(wrhs[:, :DOUT], w2)
        nc.scalar.dma_start(xtb[:, 2 * TQ : 3 * TQ], xtbd[:, 2 * TQ : 3 * TQ])
        nc.sync.dma_start(xtb[:, 3 * TQ :], xtbd[:, 3 * TQ :])

        ones_sb = const.tile([PJ, 1], bf16, tag="ones")
        nc.vector.memset(ones_sb[:], 1.0)

        # identity (epilogue transposes, bf16) + PE p-state warmup transposes
        ident0 = const.tile([PJ, PJ], f32, tag="ident0")
        masks.make_identity(nc, ident0[:])
        ident = const.tile([PJ, PJ], bf16, tag="ident")
        nc.scalar.copy(ident[:], ident0[:])

        # psum pools: hpool and num_pool own their banks from the start so
        # the h-group matmuls never wait on recycled s1b banks.
        hpool = ctx.enter_context(tc.tile_pool(name="h_psum", bufs=1, space="PSUM"))
        num_pool = ctx.enter_context(
            tc.tile_pool(name="num_psum", bufs=1, space="PSUM")
        )

        with tc.tile_pool(name="wt_psum", bufs=1, space="PSUM") as wtpool:
            warm = wtpool.tile([PJ, PJ], f32, tag="warm")
            nc.tensor.transpose(warm[:], ident0[:], ident0[:])
            nc.tensor.transpose(warm[:], ident0[:], ident0[:])
            wa12_ps = wtpool.tile([PJ, 2], f32, tag="wa12_ps")
            nc.tensor.matmul(wa12_ps[:], wt_dma[:], a12_dma[:], start=True, stop=True)
            wa1rep = const.tile([PJ, PJ], bf16, tag="wa1rep")
            nc.scalar.copy(wa1rep[:], wa12_ps[:, 0:1].broadcast_to([PJ, PJ]))
            nc.scalar.copy(wrhs[:, DOUT : DOUT + 1], wa12_ps[:, 1:2])

        # ---- hcat regions (write-once, 4 tiles each) + per-group s2 cols
        hcat = [
            big.tile([PJ, HCG * HCS], bf16, name=f"hcat{g}", tag=f"hcat{g}")
            for g in range(NJT // HCG)
        ]
        s2g = [
            big.tile([PJ, HCG], f32, name=f"s2g{g}", tag=f"s2g{g}")
            for g in range(NJT // HCG)
        ]
        for g in range(NJT // HCG):
            h3 = hcat[g][:].rearrange("p (t s) -> p t s", s=HCS)
            nc.vector.tensor_copy(
                h3[:, :, DOUT : DOUT + 1],
                ones_sb[:].broadcast_to([PJ, HCG])[:, :, None],
            )

        def emit_hgroup(g):
            h_ps = hpool.tile([PJ, HCG * (DOUT + 1)], f32, tag="hps")
            for k in range(HCG):
                jt = g * HCG + k
                base = 0 if jt < 8 else DIN
                qoff = (jt % 8) * PJ
                nc.tensor.matmul(
                    h_ps[:, k * (DOUT + 1) : (k + 1) * (DOUT + 1)],
                    xtb[base : base + DIN, qoff : qoff + PJ],
                    wrhs[base : base + DIN, :],
                    start=True, stop=True,
                )
            h4 = h_ps[:].rearrange("p (t s) -> p t s", s=DOUT + 1)
            hc3 = hcat[g][:].rearrange("p (t s) -> p t s", s=HCS)
            nc.scalar.copy(hc3[:, :, :DOUT], h4[:, :, :DOUT])
            nc.scalar.copy(s2g[g][:], h4[:, :, DOUT])

        # ---- s1b = wa1rep.T @ xtb : [128, N] f32; chunks 0/2 only need
        # the first two xtb quarters, so they go ahead of h-group 0.
        s1b_sb = big.tile([PJ, N], f32, tag="s1b")
        with tc.tile_pool(name="s1b_psum", bufs=3, space="PSUM") as spool:

            def emit_s1b(c):
                base = 0 if c < 2 else DIN
                qsl = slice((c % 2) * FCH, (c % 2) * FCH + FCH)
                s1b_ps = spool.tile([PJ, FCH], f32, name=f"s1bps{c}", tag="s1b_ps")
                nc.tensor.matmul(
                    s1b_ps[:],
                    wa1rep[base : base + DIN, :],
                    xtb[base : base + DIN, qsl],
                    start=True, stop=True,
                )
                nc.vector.tensor_copy(s1b_sb[:, c * FCH : (c + 1) * FCH], s1b_ps[:])

            emit_s1b(0)
            emit_s1b(2)
            emit_hgroup(0)
            emit_s1b(1)
            emit_s1b(3)

        # ---- mask stream: pre-emit ALL tile DMAs; the sync queue serves
        # them in order behind its xtb chunk, so no explicit gating needed.
        # Tiles 8-15 wait on buffer-rotation WAR deps automatically.
        mpool = ctx.enter_context(tc.tile_pool(name="mask", bufs=8))
        mbs = []
        for jt in range(NJT):
            mb = mpool.tile([PJ, N], u8, name=f"mb{jt}", tag="mb")
            with tc.tile_wait_until(0.0010 + 0.00005 * jt):
                nc.sync.dma_start(mb[:], mk[jt * PJ : (jt + 1) * PJ, :])
            mbs.append(mb)

        emit_hgroup(1)

        tpool = ctx.enter_context(tc.tile_pool(name="scores", bufs=3))
        ppool_e = ctx.enter_context(tc.tile_pool(name="probs", bufs=3))
        numT = [
            num_pool.tile([DOUT + 1, FCH], f32, name=f"numt{c}", tag=f"numt{c}")
            for c in range(NCH)
        ]

        # ---- main loop over j-tiles (last tile split in half-columns so
        # the score/exp/matmul tail pipelines instead of serializing)
        for jt in range(NJT):
            g, k = jt // HCG, jt % HCG
            if jt in (HCG, 2 * HCG):
                emit_hgroup(g + 1)

            mb = mbs[jt]
            lhsT = hcat[g][:, k * HCS : k * HCS + DOUT + 1]
            t_sb = tpool.tile([PJ, N], f32, tag="t")
            p_sb = ppool_e.tile([PJ, N], bf16, tag="p")

            csls = [slice(0, N)] if jt < NJT - 1 else [
                slice(0, N // 2), slice(N // 2, N)
            ]
            for csl in csls:
                nc.vector._custom_dve(
                    gat_op,
                    out=t_sb[:, csl],
                    in0=s1b_sb[:, csl],
                    in1=mb[:, csl],
                    s0=s2g[g][:, k : k + 1],
                    s1=NEG_BIG,
                    imm2=5.0,
                )
                nc.scalar.activation(p_sb[:, csl], t_sb[:, csl], AF.Exp, scale=0.2)

            for c in range(NCH):
                sl = slice(c * FCH, (c + 1) * FCH)
                nc.tensor.matmul(
                    numT[c][:], lhsT, p_sb[:, sl],
                    start=(jt == 0), stop=(jt == NJT - 1),
                )

        # ---- epilogue: per 512-chunk: drain, transpose, divide, store ----
        epool = ctx.enter_context(tc.tile_pool(name="epi", bufs=2))
        etr_pool = ctx.enter_context(
            tc.tile_pool(name="epi_psum", bufs=2, space="PSUM")
        )
        out_pool = ctx.enter_context(tc.tile_pool(name="out", bufs=1))

        out_sb = out_pool.tile([PJ, NJT * DOUT], bf16, tag="out")
        out_3d = out.rearrange("(t p) d -> p t d", p=PJ)
        out_engines = [nc.sync, nc.scalar, nc.gpsimd, nc.sync]
        TRS = DOUT + 4  # transpose slot stride (4-elem aligned for bf16 APs)
        GW = EPI_GRP * TRS
        for g in range(NJT // EPI_GRP):
            numc = epool.tile([DOUT + 1, FCH], bf16, tag="numc")
            nc.scalar.copy(numc[:], numT[g][:])

            tr_ps = etr_pool.tile([PJ, GW], bf16, tag="tr")
            for k in range(EPI_GRP):
                isl = slice(k * PJ, (k + 1) * PJ)
                nc.tensor.transpose(
                    tr_ps[:, k * TRS : k * TRS + DOUT + 1],
                    numc[:, isl],
                    ident[: DOUT + 1, : DOUT + 1],
                )
            tr_sb = epool.tile([PJ, GW], bf16, tag="tr_sb")
            trp3 = tr_ps[:].rearrange("p (k s) -> p k s", s=TRS)
            trs3 = tr_sb[:].rearrange("p (k s) -> p k s", s=TRS)
            nc.vector.tensor_copy(
                trs3[:, :, : DOUT + 1], trp3[:, :, : DOUT + 1]
            )

            tr3 = tr_sb[:].rearrange("p (k s) -> p k s", s=TRS)
            recip = epool.tile([PJ, EPI_GRP], f32, tag="recip")
            nc.vector.reciprocal(recip[:], tr3[:, :, DOUT])
            for k in range(EPI_GRP):
                it = g * EPI_GRP + k
                nc.vector.tensor_scalar_mul(
                    out_sb[:, it * DOUT : (it + 1) * DOUT],
                    tr3[:, k, :DOUT],
                    recip[:, k : k + 1],
                )
            out_engines[g].dma_start(
                out_3d[:, g * EPI_GRP : (g + 1) * EPI_GRP, :],
                out_sb[:, g * EPI_GRP * DOUT : (g + 1) * EPI_GRP * DOUT].rearrange(
                    "p (t d) -> p t d", d=DOUT
                ),
            )

    nc.compile()
    return nc


def _prep_inputs(x, adj, W, a):
    xT = np.transpose(np.asarray(x, dtype=np.float32), (0, 2, 1))  # [B, 64, N]
    xtb = np.concatenate([xT[:, :, : N // 2], xT[:, :, N // 2 :]], axis=1)
    xtb = np.ascontiguousarray(xtb).astype(ml_dtypes.bfloat16)     # [B, 128, 1024]
    mask = np.ascontiguousarray(adj.T.astype(np.uint8))
    a = np.asarray(a, dtype=np.float32)
    a12 = np.ascontiguousarray(np.stack([a[:DOUT], a[DOUT:]], axis=1))  # [64, 2]
    W = np.asarray(W, dtype=np.float32)
    w2 = np.ascontiguousarray(np.concatenate([W, W], axis=0)).astype(
        ml_dtypes.bfloat16
    )                                                               # [128, 64]
    wt2 = np.ascontiguousarray(np.concatenate([W.T, W.T], axis=1))  # [64, 128]
    in_maps = []
    for b in range(NCORES):
        in_maps.append(
            {
                "xtb": xtb[b],
                "mask": mask,
                "w2": w2,
                "wt2": wt2,
                "a12": a12,
            }
        )
    return in_maps


def kernel(x, adj, W, a):
    global _COMPILED, LAST_RESULT
    from concourse import bass_utils

    x = np.asarray(x)
    adj = np.asarray(adj)
    assert x.shape == (B, N, DIN) and adj.shape == (N, N)

    if _COMPILED is None:
        _COMPILED = _build_nc()
    nc = _COMPILED

    in_maps = _prep_inputs(x, adj, W, a)
    res = bass_utils.run_bass_kernel_spmd(
        nc,
        in_maps,
        core_ids=list(range(NCORES)),
        trace=bool(int(os.environ.get("GAT_TRACE", "0"))),
    )
    LAST_RESULT = res
    out = np.stack([res.results[c]["out"] for c in range(NCORES)], axis=0)
    return out.astype(np.float32)


# revision 18
# speedup vs baseline: 1.0944x; 1.0442x over previous
"""GAT layer (nn_GATLayer) on 8 Trainium2 NeuronCores.

Math (per batch b):
    h   = x @ W                      [N, D]
    s1  = h @ a1   (free-dim i)      [N]
    s2  = h @ a2   (partition j)     [N]
    e   = lrelu(s1_i + s2_j)  masked by adj[i, j], softmax over j
    out = attn @ h

Device formulation (per core = one batch element), in [p=j, f=i] layout:
    t[j, i]  = select(A[j, i] > 0, max(y, 5y), -1e9),  y = s1[i] + s2[j]
               (custom DVE op; A is the uint8 adjacency -> 4MB/core DMA)
    p[j, i]  = exp(0.2 * t)                      (ACT, bf16 out)
    numT[d, i] = sum_j h_cat[j, d] * p[j, i],    h_cat = [h | ones]  (bf16)
    out[i, d]  = numT[d, i] / numT[64, i]

Sharding: data-parallel over batch B=8 across the 8 cores.

Host prep: x[b].T is shipped pre-cast to bf16 in a stacked [128, 1024]
layout (both column halves vertically, so DMAs cover 128 partitions);
the adjacency as uint8 adj.T; W duplicated as [W; W] (bf16 [128, 64]) and
[W.T | W.T] (f32 [64, 128]) so base-64 partition slices of xtb have
matching-base weights; a1/a2 packed as one [64, 2] tensor.

The steady loop is DVE-bound (custom score op, 2048 cols x ~1.15ns/col).
The prologue computes wa1/wa2 with ONE matmul, s1b via a broadcast-wa1
matmul straight from xtb, and each j-tile's h/s2 with one matmul against
[W | wa2], drained per group-of-4 into write-once hcat regions. The last
tile is split into halves so the exp/matmul tail pipelines.
"""

import os
import sys

sys.path.insert(0, "/opt/trn_rl_repo")

import numpy as np
import ml_dtypes

B, N, DIN, DOUT = 8, 2048, 64, 64
NCORES = 8
PJ = 128              # j-tile partition size
NJT = N // PJ         # 16 j-tiles
FCH = 512             # psum bank chunk (fp32)
NCH = N // FCH        # 4 chunks of the free dim
XQ = N // 2           # stacked xtb layout: [128, 1024]
NEG_BIG = -1.0e9
HCG = 4               # h/s2 tiles per psum group / hcat region
HCS = 66              # hcat stride: 64 h cols + 1 ones col + 1 pad
EPI_GRP = 4           # epilogue transposes packed per psum bank tile

_GAT_OP = None
_COMPILED = None
LAST_RESULT = None    # BassKernelResults from the last run (for test.py)


def _register_gat_op():
    """Fused score op: out = select(Src1 > 0, max(y, y*imm2), C1), y = Src0+C0.

    in0 = s1 broadcast [128, N] (f32), s0 = s2 per-partition [128, 1] (f32),
    in1 = adjacency tile [128, N] (uint8 0/1), s1 = -1e9, imm2 = 5.0.
    lrelu(x) = 0.2*max(5x, x); exp(0.2 * -1e9) -> 0 for masked entries.
    """
    global _GAT_OP
    if _GAT_OP is not None:
        return _GAT_OP
    from concourse.dve_ops import (
        OPS,
        CUSTOM_DVE_SPECS,
        DveOp,
        _SUB_OPCODE_FOR_NAME,
    )
    from concourse.dve_spec import (
        Spec, Src0, Src1, C0, C1, C2, Zero, maxx, select, lower, _has_src1,
    )
    from concourse.dve_uop import DveOpSpec

    name = "GAT_SCORE_U8_ANT"
    if name in _SUB_OPCODE_FOR_NAME:
        _GAT_OP = next(op for op in OPS if op.name == name)
        return _GAT_OP

    _y = Src0 + C0
    body = select(Src1 > Zero, maxx(_y, _y * C2), C1)

    def _ref(in0, in1, s0, s1, imm2):
        y = in0.astype(np.float32) + s0
        t = np.maximum(y, y * imm2)
        return np.where(in1.astype(np.float32) > 0.0, t, s1).astype(np.float32)

    spec = Spec(body=body, reference=_ref)
    row = max(_SUB_OPCODE_FOR_NAME.values()) + 1
    assert row < 0x20
    _SUB_OPCODE_FOR_NAME[name] = row
    shas = {}
    for ver in ("v3", "v4"):
        tmp = DveOpSpec(
            name=name, opcode=row, uops=lower(spec, ver=ver), rd1_en=_has_src1(spec)
        )
        shas[ver] = tmp.sha(ver)
    op = DveOp(name, spec, subdim=False, uops_sha=shas)
    OPS.append(op)
    CUSTOM_DVE_SPECS[name] = spec
    _GAT_OP = op
    return op


def _build_nc():
    """Build the Bass module (shared SPMD program for all 8 cores)."""
    from contextlib import ExitStack

    import concourse.bass as bass
    import concourse.tile as tile
    from concourse import bacc, masks, mybir

    gat_op = _register_gat_op()

    f32 = mybir.dt.float32
    bf16 = mybir.dt.bfloat16
    u8 = mybir.dt.uint8
    AF = mybir.ActivationFunctionType

    nc = bacc.Bacc("TRN2", target_bir_lowering=False, debug=False, num_devices=NCORES)

    # stacked bf16 x: rows 0-63 = x.T cols 0:1024, rows 64-127 = cols 1024:2048
    xtbd = nc.dram_tensor("xtb", [PJ, XQ], bf16, kind="ExternalInput").ap()
    mk = nc.dram_tensor("mask", [N, N], u8, kind="ExternalInput").ap()
    w2 = nc.dram_tensor("w2", [PJ, DOUT], bf16, kind="ExternalInput").ap()   # [W; W]
    wa12 = nc.dram_tensor("wa12", [PJ, 2], f32, kind="ExternalInput").ap()   # [W@a1 | W@a2] x2
    out = nc.dram_tensor("out", [N, DOUT], bf16, kind="ExternalOutput").ap()

    with ExitStack() as ctx:
        tc = ctx.enter_context(tile.TileContext(nc))

        const = ctx.enter_context(tc.tile_pool(name="const", bufs=1))
        big = ctx.enter_context(tc.tile_pool(name="big", bufs=1))

        # ---- input DMAs: one xtb chunk per dispatcher queue (each queue
        # serves its own list in order, striped over the 16 DMA engines).
        # The mask stream follows on the sync queue behind its xtb chunk.
        xtb = const.tile([PJ, XQ], bf16, tag="xtb")
        wrhs = const.tile([PJ, DOUT + 1], bf16, tag="wrhs")
        wa12_dma = const.tile([PJ, 2], f32, tag="wa12d")
        TQ = XQ // 4  # 256-col quarters
        nc.sync.dma_start(wa12_dma[:], wa12)
        nc.scalar.dma_start(xtb[:, 0:TQ], xtbd[:, 0:TQ])
        nc.gpsimd.dma_start(xtb[:, TQ : 2 * TQ], xtbd[:, TQ : 2 * TQ])
        nc.scalar.dma_start(wrhs[:, :DOUT], w2)
        nc.scalar.dma_start(xtb[:, 2 * TQ : 3 * TQ], xtbd[:, 2 * TQ : 3 * TQ])
        nc.sync.dma_start(xtb[:, 3 * TQ :], xtbd[:, 3 * TQ :])

        ones_sb = const.tile([PJ, 1], bf16, tag="ones")
        nc.vector.memset(ones_sb[:], 1.0)

        # identity (epilogue transposes, bf16) + PE p-state warmup transposes
        ident0 = const.tile([PJ, PJ], f32, tag="ident0")
        masks.make_identity(nc, ident0[:])
        ident = const.tile([PJ, PJ], bf16, tag="ident")
        nc.scalar.copy(ident[:], ident0[:])

        # psum pools: hpool and num_pool own their banks from the start so
        # the h-group matmuls never wait on recycled s1b banks.
        hpool = ctx.enter_context(tc.tile_pool(name="h_psum", bufs=1, space="PSUM"))
        num_pool = ctx.enter_context(
            tc.tile_pool(name="num_psum", bufs=1, space="PSUM")
        )

        with tc.tile_pool(name="wt_psum", bufs=1, space="PSUM") as wtpool:
            warm = wtpool.tile([PJ, PJ], f32, tag="warm")
            nc.tensor.transpose(warm[:], ident0[:], ident0[:])
            nc.tensor.transpose(warm[:], ident0[:], ident0[:])
        wa1rep = const.tile([PJ, PJ], bf16, tag="wa1rep")
        nc.scalar.copy(wa1rep[:], wa12_dma[:, 0:1].broadcast_to([PJ, PJ]))
        nc.scalar.copy(wrhs[:, DOUT : DOUT + 1], wa12_dma[:, 1:2])

        # ---- hcat regions (write-once, 4 tiles each) + per-group s2 cols
        hcat = [
            big.tile([PJ, HCG * HCS], bf16, name=f"hcat{g}", tag=f"hcat{g}")
            for g in range(NJT // HCG)
        ]
        s2g = [
            big.tile([PJ, HCG], f32, name=f"s2g{g}", tag=f"s2g{g}")
            for g in range(NJT // HCG)
        ]
        for g in range(NJT // HCG):
            h3 = hcat[g][:].rearrange("p (t s) -> p t s", s=HCS)
            nc.vector.tensor_copy(
                h3[:, :, DOUT : DOUT + 1],
                ones_sb[:].broadcast_to([PJ, HCG])[:, :, None],
            )

        def emit_hgroup(g):
            h_ps = hpool.tile([PJ, HCG * (DOUT + 1)], f32, tag="hps")
            for k in range(HCG):
                jt = g * HCG + k
                base = 0 if jt < 8 else DIN
                qoff = (jt % 8) * PJ
                nc.tensor.matmul(
                    h_ps[:, k * (DOUT + 1) : (k + 1) * (DOUT + 1)],
                    xtb[base : base + DIN, qoff : qoff + PJ],
                    wrhs[base : base + DIN, :],
                    start=True, stop=True,
                )
            h4 = h_ps[:].rearrange("p (t s) -> p t s", s=DOUT + 1)
            hc3 = hcat[g][:].rearrange("p (t s) -> p t s", s=HCS)
            nc.scalar.copy(hc3[:, :, :DOUT], h4[:, :, :DOUT])
            nc.scalar.copy(s2g[g][:], h4[:, :, DOUT])

        # ---- s1b = wa1rep.T @ xtb : [128, N] f32; chunks 0/2 only need
        # the first two xtb quarters, so they go ahead of h-group 0.
        s1b_sb = big.tile([PJ, N], f32, tag="s1b")
        with tc.tile_pool(name="s1b_psum", bufs=3, space="PSUM") as spool:

            def emit_s1b(c):
                base = 0 if c < 2 else DIN
                qsl = slice((c % 2) * FCH, (c % 2) * FCH + FCH)
                s1b_ps = spool.tile([PJ, FCH], f32, name=f"s1bps{c}", tag="s1b_ps")
                nc.tensor.matmul(
                    s1b_ps[:],
                    wa1rep[base : base + DIN, :],
                    xtb[base : base + DIN, qsl],
                    start=True, stop=True,
                )
                nc.vector.tensor_copy(s1b_sb[:, c * FCH : (c + 1) * FCH], s1b_ps[:])

            emit_s1b(0)
            emit_s1b(2)
            emit_hgroup(0)
            emit_s1b(1)
            emit_s1b(3)

        # ---- mask stream: pre-emit ALL tile DMAs; the sync queue serves
        # them in order behind its xtb chunk, so no explicit gating needed.
        # Tiles 8-15 wait on buffer-rotation WAR deps automatically.
        mpool = ctx.enter_context(tc.tile_pool(name="mask", bufs=8))
        mbs = []
        for jt in range(NJT):
            mb = mpool.tile([PJ, N], u8, name=f"mb{jt}", tag="mb")
            with tc.tile_wait_until(0.0010 + 0.00005 * jt):
                nc.sync.dma_start(mb[:], mk[jt * PJ : (jt + 1) * PJ, :])
            mbs.append(mb)

        emit_hgroup(1)

        tpool = ctx.enter_context(tc.tile_pool(name="scores", bufs=3))
        ppool_e = ctx.enter_context(tc.tile_pool(name="probs", bufs=3))
        numT = [
            num_pool.tile([DOUT + 1, FCH], f32, name=f"numt{c}", tag=f"numt{c}")
            for c in range(NCH)
        ]

        # ---- main loop over j-tiles (last tile split in half-columns so
        # the score/exp/matmul tail pipelines instead of serializing)
        for jt in range(NJT):
            g, k = jt // HCG, jt % HCG
            if jt in (HCG, 2 * HCG):
                emit_hgroup(g + 1)

            mb = mbs[jt]
            lhsT = hcat[g][:, k * HCS : k * HCS + DOUT + 1]
            t_sb = tpool.tile([PJ, N], f32, tag="t")
            p_sb = ppool_e.tile([PJ, N], bf16, tag="p")

            csls = [slice(0, N)] if jt < NJT - 1 else [
                slice(0, N // 2), slice(N // 2, N)
            ]
            for csl in csls:
                nc.vector._custom_dve(
                    gat_op,
                    out=t_sb[:, csl],
                    in0=s1b_sb[:, csl],
                    in1=mb[:, csl],
                    s0=s2g[g][:, k : k + 1],
                    s1=NEG_BIG,
                    imm2=5.0,
                )
                nc.scalar.activation(p_sb[:, csl], t_sb[:, csl], AF.Exp, scale=0.2)

            for c in range(NCH):
                sl = slice(c * FCH, (c + 1) * FCH)
                nc.tensor.matmul(
                    numT[c][:], lhsT, p_sb[:, sl],
                    start=(jt == 0), stop=(jt == NJT - 1),
                )

        # ---- epilogue: per 512-chunk: drain, transpose, divide, store ----
        epool = ctx.enter_context(tc.tile_pool(name="epi", bufs=2))
        etr_pool = ctx.enter_context(
            tc.tile_pool(name="epi_psum", bufs=2, space="PSUM")
        )
        out_pool = ctx.enter_context(tc.tile_pool(name="out", bufs=1))

        out_sb = out_pool.tile([PJ, NJT * DOUT], bf16, tag="out")
        out_3d = out.rearrange("(t p) d -> p t d", p=PJ)
        out_engines = [nc.sync, nc.scalar, nc.gpsimd, nc.sync]
        TRS = DOUT + 4  # transpose slot stride (4-elem aligned for bf16 APs)
        GW = EPI_GRP * TRS
        for g in range(NJT // EPI_GRP):
            numc = epool.tile([DOUT + 1, FCH], bf16, tag="numc")
            if g < 3:
                nc.scalar.copy(numc[:], numT[g][:])
            else:
                nc.vector.tensor_copy(numc[:], numT[g][:])

            tr_ps = etr_pool.tile([PJ, GW], bf16, tag="tr")
            for k in range(EPI_GRP):
                isl = slice(k * PJ, (k + 1) * PJ)
                nc.tensor.transpose(
                    tr_ps[:, k * TRS : k * TRS + DOUT + 1],
                    numc[:, isl],
                    ident[: DOUT + 1, : DOUT + 1],
                )
            tr_sb = epool.tile([PJ, GW], bf16, tag="tr_sb")
            trp3 = tr_ps[:].rearrange("p (k s) -> p k s", s=TRS)
            trs3 = tr_sb[:].rearrange("p (k s) -> p k s", s=TRS)
            nc.scalar.copy(trs3[:, :, : DOUT + 1], trp3[:, :, : DOUT + 1])

            tr3 = tr_sb[:].rearrange("p (k s) -> p k s", s=TRS)
            recip = epool.tile([PJ, EPI_GRP], f32, tag="recip")
            nc.vector.reciprocal(recip[:], tr3[:, :, DOUT])
            for k in range(EPI_GRP):
                it = g * EPI_GRP + k
                nc.vector.tensor_scalar_mul(
                    out_sb[:, it * DOUT : (it + 1) * DOUT],
                    tr3[:, k, :DOUT],
                    recip[:, k : k + 1],
                )
            out_engines[g].dma_start(
                out_3d[:, g * EPI_GRP : (g + 1) * EPI_GRP, :],
                out_sb[:, g * EPI_GRP * DOUT : (g + 1) * EPI_GRP * DOUT].rearrange(
                    "p (t d) -> p t d", d=DOUT
                ),
            )

    nc.compile()
    return nc


def _prep_inputs(x, adj, W, a):
    xT = np.transpose(np.asarray(x, dtype=np.float32), (0, 2, 1))  # [B, 64, N]
    xtb = np.concatenate([xT[:, :, : N // 2], xT[:, :, N // 2 :]], axis=1)
    xtb = np.ascontiguousarray(xtb).astype(ml_dtypes.bfloat16)     # [B, 128, 1024]
    mask = np.ascontiguousarray(adj.T.astype(np.uint8))
    a = np.asarray(a, dtype=np.float32)
    W = np.asarray(W, dtype=np.float32)
    w2 = np.ascontiguousarray(np.concatenate([W, W], axis=0)).astype(
        ml_dtypes.bfloat16
    )                                                               # [128, 64]
    wa = W @ np.stack([a[:DOUT], a[DOUT:]], axis=1)                 # [64, 2]
    wa12 = np.ascontiguousarray(np.concatenate([wa, wa], axis=0), dtype=np.float32)
    in_maps = []
    for b in range(NCORES):
        in_maps.append(
            {
                "xtb": xtb[b],
                "mask": mask,
                "w2": w2,
                "wa12": wa12,
            }
        )
    return in_maps


def kernel(x, adj, W, a):
    global _COMPILED, LAST_RESULT
    from concourse import bass_utils

    x = np.asarray(x)
    adj = np.asarray(adj)
    assert x.shape == (B, N, DIN) and adj.shape == (N, N)

    if _COMPILED is None:
        _COMPILED = _build_nc()
    nc = _COMPILED

    in_maps = _prep_inputs(x, adj, W, a)
    res = bass_utils.run_bass_kernel_spmd(
        nc,
        in_maps,
        core_ids=list(range(NCORES)),
        trace=bool(int(os.environ.get("GAT_TRACE", "0"))),
    )
    LAST_RESULT = res
    out = np.stack([res.results[c]["out"] for c in range(NCORES)], axis=0)
    return out.astype(np.float32)


# revision 20
# speedup vs baseline: 1.1305x; 1.0331x over previous
"""GAT layer (nn_GATLayer) on 8 Trainium2 NeuronCores.

Math (per batch b):
    h   = x @ W                      [N, D]
    s1  = h @ a1   (free-dim i)      [N]
    s2  = h @ a2   (partition j)     [N]
    e   = lrelu(s1_i + s2_j)  masked by adj[i, j], softmax over j
    out = attn @ h

Device formulation (per core = one batch element), in [p=j, f=i] layout:
    t[j, i]  = select(A[j, i] > 0, max(y, 5y), -1e9),  y = s1[i] + s2[j]
               (custom DVE op; A is the uint8 adjacency -> 4MB/core DMA)
    p[j, i]  = exp(0.2 * t)                      (ACT, bf16 out)
    numT[d, i] = sum_j h_cat[j, d] * p[j, i],    h_cat = [h | ones]  (bf16)
    out[i, d]  = numT[d, i] / numT[64, i]

Sharding: data-parallel over batch B=8 across the 8 cores.

Host prep: x[b].T is shipped pre-cast to bf16 in a stacked [128, 1024]
layout (both column halves vertically, so DMAs cover 128 partitions);
the adjacency as uint8 adj.T; W duplicated as [W; W] (bf16 [128, 64]) and
[W.T | W.T] (f32 [64, 128]) so base-64 partition slices of xtb have
matching-base weights; a1/a2 packed as one [64, 2] tensor.

The steady loop is DVE-bound (custom score op, 2048 cols x ~1.15ns/col).
The prologue computes wa1/wa2 with ONE matmul, s1b via a broadcast-wa1
matmul straight from xtb, and each j-tile's h/s2 with one matmul against
[W | wa2], drained per group-of-4 into write-once hcat regions. The last
tile is split into halves so the exp/matmul tail pipelines.
"""

import os
import sys

sys.path.insert(0, "/opt/trn_rl_repo")

import numpy as np
import ml_dtypes

B, N, DIN, DOUT = 8, 2048, 64, 64
NCORES = 8
PJ = 128              # j-tile partition size
NJT = N // PJ         # 16 j-tiles
FCH = 512             # psum bank chunk (fp32)
NCH = N // FCH        # 4 chunks of the free dim
XQ = N // 2           # stacked xtb layout: [128, 1024]
NEG_BIG = -1.0e9
HCG = 4               # h/s2 tiles per psum group / hcat region
HCS = 66              # hcat stride: 64 h cols + 1 ones col + 1 pad
EPI_GRP = 4           # epilogue transposes packed per psum bank tile

_GAT_OP = None
_COMPILED = None
LAST_RESULT = None    # BassKernelResults from the last run (for test.py)


def _register_gat_op():
    """Fused score op: out = select(Src1 > 0, max(y, y*imm2), C1), y = Src0+C0.

    in0 = s1 broadcast [128, N] (f32), s0 = s2 per-partition [128, 1] (f32),
    in1 = adjacency tile [128, N] (uint8 0/1), s1 = -1e9, imm2 = 5.0.
    lrelu(x) = 0.2*max(5x, x); exp(0.2 * -1e9) -> 0 for masked entries.
    """
    global _GAT_OP
    if _GAT_OP is not None:
        return _GAT_OP
    from concourse.dve_ops import (
        OPS,
        CUSTOM_DVE_SPECS,
        DveOp,
        _SUB_OPCODE_FOR_NAME,
    )
    from concourse.dve_spec import (
        Spec, Src0, Src1, C0, C1, C2, Zero, maxx, select, lower, _has_src1,
    )
    from concourse.dve_uop import DveOpSpec

    name = "GAT_SCORE_U8_ANT"
    if name in _SUB_OPCODE_FOR_NAME:
        _GAT_OP = next(op for op in OPS if op.name == name)
        return _GAT_OP

    _y = Src0 + C0
    body = select(Src1 > Zero, maxx(_y, _y * C2), C1)

    def _ref(in0, in1, s0, s1, imm2):
        y = in0.astype(np.float32) + s0
        t = np.maximum(y, y * imm2)
        return np.where(in1.astype(np.float32) > 0.0, t, s1).astype(np.float32)

    spec = Spec(body=body, reference=_ref)
    row = max(_SUB_OPCODE_FOR_NAME.values()) + 1
    assert row < 0x20
    _SUB_OPCODE_FOR_NAME[name] = row
    shas = {}
    for ver in ("v3", "v4"):
        tmp = DveOpSpec(
            name=name, opcode=row, uops=lower(spec, ver=ver), rd1_en=_has_src1(spec)
        )
        shas[ver] = tmp.sha(ver)
    op = DveOp(name, spec, subdim=False, uops_sha=shas)
    OPS.append(op)
    CUSTOM_DVE_SPECS[name] = spec
    _GAT_OP = op
    return op


def _build_nc():
    """Build the Bass module (shared SPMD program for all 8 cores)."""
    from contextlib import ExitStack

    import concourse.bass as bass
    import concourse.tile as tile
    from concourse import bacc, masks, mybir

    gat_op = _register_gat_op()

    f32 = mybir.dt.float32
    bf16 = mybir.dt.bfloat16
    u8 = mybir.dt.uint8
    AF = mybir.ActivationFunctionType

    nc = bacc.Bacc("TRN2", target_bir_lowering=False, debug=False, num_devices=NCORES)

    # stacked bf16 x: rows 0-63 = x.T cols 0:1024, rows 64-127 = cols 1024:2048
    xtbd = nc.dram_tensor("xtb", [PJ, XQ], bf16, kind="ExternalInput").ap()
    mk = nc.dram_tensor("mask", [N, N], u8, kind="ExternalInput").ap()
    w2 = nc.dram_tensor("w2", [PJ, DOUT], bf16, kind="ExternalInput").ap()   # [W; W]
    wa12 = nc.dram_tensor("wa12", [PJ, 2], f32, kind="ExternalInput").ap()   # [W@a1 | W@a2] x2
    out = nc.dram_tensor("out", [N, DOUT], bf16, kind="ExternalOutput").ap()

    with ExitStack() as ctx:
        tc = ctx.enter_context(tile.TileContext(nc))

        const = ctx.enter_context(tc.tile_pool(name="const", bufs=1))
        big = ctx.enter_context(tc.tile_pool(name="big", bufs=1))

        # ---- input DMAs: one xtb chunk per dispatcher queue (each queue
        # serves its own list in order, striped over the 16 DMA engines).
        # The mask stream follows on the sync queue behind its xtb chunk.
        xtb = const.tile([PJ, XQ], bf16, tag="xtb")
        wrhs = const.tile([PJ, DOUT + 1], bf16, tag="wrhs")
        wa12_dma = const.tile([PJ, 2], f32, tag="wa12d")
        TQ = XQ // 4  # 256-col quarters
        nc.sync.dma_start(wa12_dma[:], wa12)
        nc.scalar.dma_start(xtb[:, 0:TQ], xtbd[:, 0:TQ])
        nc.gpsimd.dma_start(xtb[:, TQ : 2 * TQ], xtbd[:, TQ : 2 * TQ])
        nc.gpsimd.dma_start(xtb[:, 2 * TQ : 3 * TQ], xtbd[:, 2 * TQ : 3 * TQ])
        nc.sync.dma_start(xtb[:, 3 * TQ :], xtbd[:, 3 * TQ :])
        nc.scalar.dma_start(wrhs[:, :DOUT], w2)

        ones_sb = const.tile([PJ, 1], bf16, tag="ones")
        nc.vector.memset(ones_sb[:], 1.0)

        # identity (epilogue transposes, bf16) + PE p-state warmup transposes
        ident0 = const.tile([PJ, PJ], f32, tag="ident0")
        masks.make_identity(nc, ident0[:])
        ident = const.tile([PJ, PJ], bf16, tag="ident")
        nc.scalar.copy(ident[:], ident0[:])

        # psum pools: hpool and num_pool own their banks from the start so
        # the h-group matmuls never wait on recycled s1b banks.
        hpool = ctx.enter_context(tc.tile_pool(name="h_psum", bufs=1, space="PSUM"))
        num_pool = ctx.enter_context(
            tc.tile_pool(name="num_psum", bufs=1, space="PSUM")
        )

        with tc.tile_pool(name="wt_psum", bufs=1, space="PSUM") as wtpool:
            warm = wtpool.tile([PJ, PJ], f32, tag="warm")
            nc.tensor.transpose(warm[:], ident0[:], ident0[:])
            nc.tensor.transpose(warm[:], ident0[:], ident0[:])
        wa1rep = const.tile([PJ, PJ], bf16, tag="wa1rep")
        nc.scalar.copy(wa1rep[:], wa12_dma[:, 0:1].broadcast_to([PJ, PJ]))
        nc.scalar.copy(wrhs[:, DOUT : DOUT + 1], wa12_dma[:, 1:2])

        # ---- hcat regions (write-once, 4 tiles each) + per-group s2 cols
        hcat = [
            big.tile([PJ, HCG * HCS], bf16, name=f"hcat{g}", tag=f"hcat{g}")
            for g in range(NJT // HCG)
        ]
        s2g = [
            big.tile([PJ, HCG], f32, name=f"s2g{g}", tag=f"s2g{g}")
            for g in range(NJT // HCG)
        ]
        for g in range(NJT // HCG):
            h3 = hcat[g][:].rearrange("p (t s) -> p t s", s=HCS)
            nc.vector.tensor_copy(
                h3[:, :, DOUT : DOUT + 1],
                ones_sb[:].broadcast_to([PJ, HCG])[:, :, None],
            )

        def emit_hgroup(g):
            h_ps = hpool.tile([PJ, HCG * (DOUT + 1)], f32, tag="hps")
            for k in range(HCG):
                jt = g * HCG + k
                base = 0 if jt < 8 else DIN
                qoff = (jt % 8) * PJ
                nc.tensor.matmul(
                    h_ps[:, k * (DOUT + 1) : (k + 1) * (DOUT + 1)],
                    xtb[base : base + DIN, qoff : qoff + PJ],
                    wrhs[base : base + DIN, :],
                    start=True, stop=True,
                )
            h4 = h_ps[:].rearrange("p (t s) -> p t s", s=DOUT + 1)
            hc3 = hcat[g][:].rearrange("p (t s) -> p t s", s=HCS)
            nc.scalar.copy(hc3[:, :, :DOUT], h4[:, :, :DOUT])
            nc.scalar.copy(s2g[g][:], h4[:, :, DOUT])

        # ---- s1b = wa1rep.T @ xtb : [128, N] f32; chunks 0/2 only need
        # the first two xtb quarters, so they go ahead of h-group 0.
        s1b_sb = big.tile([PJ, N], f32, tag="s1b")
        with tc.tile_pool(name="s1b_psum", bufs=3, space="PSUM") as spool:

            def emit_s1b(c):
                base = 0 if c < 2 else DIN
                qsl = slice((c % 2) * FCH, (c % 2) * FCH + FCH)
                s1b_ps = spool.tile([PJ, FCH], f32, name=f"s1bps{c}", tag="s1b_ps")
                nc.tensor.matmul(
                    s1b_ps[:],
                    wa1rep[base : base + DIN, :],
                    xtb[base : base + DIN, qsl],
                    start=True, stop=True,
                )
                nc.vector.tensor_copy(s1b_sb[:, c * FCH : (c + 1) * FCH], s1b_ps[:])

            emit_s1b(0)
            emit_s1b(2)
            emit_hgroup(0)
            emit_s1b(1)
            emit_s1b(3)

        # ---- mask stream: pre-emit ALL tile DMAs; the sync queue serves
        # them in order behind its xtb chunk, so no explicit gating needed.
        # Tiles 8-15 wait on buffer-rotation WAR deps automatically.
        mpool = ctx.enter_context(tc.tile_pool(name="mask", bufs=8))
        mbs = []
        for jt in range(NJT):
            mb = mpool.tile([PJ, N], u8, name=f"mb{jt}", tag="mb")
            with tc.tile_wait_until(0.0010 + 0.00005 * jt):
                nc.sync.dma_start(mb[:], mk[jt * PJ : (jt + 1) * PJ, :])
            mbs.append(mb)

        emit_hgroup(1)

        tpool = ctx.enter_context(tc.tile_pool(name="scores", bufs=3))
        ppool_e = ctx.enter_context(tc.tile_pool(name="probs", bufs=3))
        numT = [
            num_pool.tile([DOUT + 1, FCH], f32, name=f"numt{c}", tag=f"numt{c}")
            for c in range(NCH)
        ]

        # ---- main loop over j-tiles (last tile split in half-columns so
        # the score/exp/matmul tail pipelines instead of serializing)
        for jt in range(NJT):
            g, k = jt // HCG, jt % HCG
            if jt in (HCG, 2 * HCG):
                emit_hgroup(g + 1)

            mb = mbs[jt]
            lhsT = hcat[g][:, k * HCS : k * HCS + DOUT + 1]
            t_sb = tpool.tile([PJ, N], f32, tag="t")
            p_sb = ppool_e.tile([PJ, N], bf16, tag="p")

            csls = [slice(0, N)] if jt < NJT - 1 else [
                slice(0, N // 2), slice(N // 2, N)
            ]
            for csl in csls:
                nc.vector._custom_dve(
                    gat_op,
                    out=t_sb[:, csl],
                    in0=s1b_sb[:, csl],
                    in1=mb[:, csl],
                    s0=s2g[g][:, k : k + 1],
                    s1=NEG_BIG,
                    imm2=5.0,
                )
                nc.scalar.activation(p_sb[:, csl], t_sb[:, csl], AF.Exp, scale=0.2)

            for c in range(NCH):
                sl = slice(c * FCH, (c + 1) * FCH)
                nc.tensor.matmul(
                    numT[c][:], lhsT, p_sb[:, sl],
                    start=(jt == 0), stop=(jt == NJT - 1),
                )

        # ---- epilogue: per 512-chunk: drain, transpose, divide, store ----
        epool = ctx.enter_context(tc.tile_pool(name="epi", bufs=2))
        etr_pool = ctx.enter_context(
            tc.tile_pool(name="epi_psum", bufs=2, space="PSUM")
        )
        out_pool = ctx.enter_context(tc.tile_pool(name="out", bufs=1))

        out_sb = out_pool.tile([PJ, NJT * DOUT], bf16, tag="out")
        out_3d = out.rearrange("(t p) d -> p t d", p=PJ)
        out_engines = [nc.sync, nc.scalar, nc.gpsimd, nc.sync]
        TRS = DOUT + 4  # transpose slot stride (4-elem aligned for bf16 APs)
        GW = EPI_GRP * TRS

        # pass 1: drain all numT chunks (ACT takes 0-2, DVE takes 3 so the
        # last chunk doesn't queue behind the ACT chain)
        numcs = []
        for g in range(NJT // EPI_GRP):
            numc = epool.tile([DOUT + 1, FCH], bf16, name=f"numc{g}", tag="numc")
            if g < 3:
                nc.scalar.copy(numc[:], numT[g][:])
            else:
                nc.vector.tensor_copy(numc[:], numT[g][:])
            numcs.append(numc)

        # pass 2: per group: transpose, drain, reciprocal, divide, store
        for g in range(NJT // EPI_GRP):
            numc = numcs[g]
            tr_ps = etr_pool.tile([PJ, GW], bf16, tag="tr")
            for k in range(EPI_GRP):
                isl = slice(k * PJ, (k + 1) * PJ)
                nc.tensor.transpose(
                    tr_ps[:, k * TRS : k * TRS + DOUT + 1],
                    numc[:, isl],
                    ident[: DOUT + 1, : DOUT + 1],
                )
            tr_sb = epool.tile([PJ, GW], bf16, tag="tr_sb")
            trp3 = tr_ps[:].rearrange("p (k s) -> p k s", s=TRS)
            trs3 = tr_sb[:].rearrange("p (k s) -> p k s", s=TRS)
            nc.scalar.copy(trs3[:, :, : DOUT + 1], trp3[:, :, : DOUT + 1])

            recip = epool.tile([PJ, EPI_GRP], bf16, tag="recip")
            with nc.allow_low_precision(reason="bf16 epilogue divide, <1% err"):
                nc.vector.reciprocal(recip[:], trs3[:, :, DOUT])
            osl = out_sb[:, g * EPI_GRP * DOUT : (g + 1) * EPI_GRP * DOUT]
            nc.vector.tensor_tensor(
                osl.rearrange("p (k d) -> p k d", d=DOUT),
                trs3[:, :, :DOUT],
                recip[:, :, None].broadcast_to([PJ, EPI_GRP, DOUT]),
                op=mybir.AluOpType.mult,
            )
            out_engines[g].dma_start(
                out_3d[:, g * EPI_GRP : (g + 1) * EPI_GRP, :],
                osl.rearrange("p (t d) -> p t d", d=DOUT),
            )

    nc.compile()
    return nc


def _prep_inputs(x, adj, W, a):
    xT = np.transpose(np.asarray(x, dtype=np.float32), (0, 2, 1))  # [B, 64, N]
    xtb = np.concatenate([xT[:, :, : N // 2], xT[:, :, N // 2 :]], axis=1)
    xtb = np.ascontiguousarray(xtb).astype(ml_dtypes.bfloat16)     # [B, 128, 1024]
    mask = np.ascontiguousarray(adj.T.astype(np.uint8))
    a = np.asarray(a, dtype=np.float32)
    W = np.asarray(W, dtype=np.float32)
    w2 = np.ascontiguousarray(np.concatenate([W, W], axis=0)).astype(
        ml_dtypes.bfloat16
    )                                                               # [128, 64]
    wa = W @ np.stack([a[:DOUT], a[DOUT:]], axis=1)                 # [64, 2]
    wa12 = np.ascontiguousarray(np.concatenate([wa, wa], axis=0), dtype=np.float32)
    in_maps = []
    for b in range(NCORES):
        in_maps.append(
            {
                "xtb": xtb[b],
                "mask": mask,
                "w2": w2,
                "wa12": wa12,
            }
        )
    return in_maps


def kernel(x, adj, W, a):
    global _COMPILED, LAST_RESULT
    from concourse import bass_utils

    x = np.asarray(x)
    adj = np.asarray(adj)
    assert x.shape == (B, N, DIN) and adj.shape == (N, N)

    if _COMPILED is None:
        _COMPILED = _build_nc()
    nc = _COMPILED

    in_maps = _prep_inputs(x, adj, W, a)
    res = bass_utils.run_bass_kernel_spmd(
        nc,
        in_maps,
        core_ids=list(range(NCORES)),
        trace=bool(int(os.environ.get("GAT_TRACE", "0"))),
    )
    LAST_RESULT = res
    out = np.stack([res.results[c]["out"] for c in range(NCORES)], axis=0)
    return out.astype(np.float32)
